# revision 10
# baseline (speedup 1.0000x reference)
"""DynamicChunker Trainium2 kernel.

Strategy
--------
The reference runs a sequential scan over chunk starts, but each scan step is a
pure function of the window start position s. We therefore evaluate ALL
B*T = 128 possible starts x K=16 candidate lengths in parallel on 8 NeuronCores
(16 starts per core), and do the trivial pointer-chasing chunk chain on host.

Per (start, length) only the cls row (query 0) of the tiny transformer matters:
  - scores for the cls query fold to one matvec:  s_h[k] = x_k . (SCALE*Wk_h^T q0_h) + c_h
  - prefix softmax over candidate lengths comes from cumulative sums of
    e = exp(s) and e*V along the window axis (all 16 lengths in one pass).
  - out-proj -> LN -> FF(2048) -> LN -> classifier evaluated for the cls row
    only, batched as 256 columns per core (16 starts x 16 lengths).

Everything on device is laid out feature-major: SBUF tiles [128 partitions =
feature chunk, 256 free = (token j | length L)*16 + local problem]. All matmuls
contract over feature chunks of 128.
"""

import numpy as np

import concourse.bass as bass
import concourse.bacc as bacc
import concourse.mybir as mybir
import concourse.tile as tile
from concourse.bass_utils import run_bass_kernel_spmd

F32 = mybir.dt.float32
AF = mybir.ActivationFunctionType
ALU = mybir.AluOpType

B, T, D, K, H, FFD = 2, 64, 256, 16, 4, 2048
HD = D // H
EPS = 1e-5
THR = 0.5
SCALE = 1.0 / float(np.sqrt(HD))
NCORES = 8
PPC = (B * T) // NCORES          # problems (starts) per core = 16
NCOL = PPC * K                   # 256 columns per core
NCH = D // 128                   # feature chunks = 2
NF = FFD // 128                  # ff chunks = 16

# packed per-partition scalar columns in `scal` [128, 18]
S_BOCLS = 0    # bo + cls          (2 cols)
S_BV = 2       # bv                (2)
S_LN1W = 4     # (2)
S_LN1B = 6     # (2)
S_LN2W = 8     # (2)
S_LN2B = 10    # (2)
S_B2 = 12      # (2)
S_C1 = 14      # clf1_b            (2)
S_INIT = 16    # e0*v_cls          (2)
NSCAL = 18


def _build_nc(c2_const: float):
    nc = bacc.Bacc(None, target_bir_lowering=False, debug=False)

    xt_d = nc.declare_dram_parameter("xt", [128, NCH, NCOL], F32, isOutput=False)
    wtilde_d = nc.declare_dram_parameter("wtilde", [128, NCH, H], F32, isOutput=False)
    sm4_d = nc.declare_dram_parameter("sm4", [H, 2], F32, isOutput=False)
    hb_d = nc.declare_dram_parameter("hb", [H, NCH, 128], F32, isOutput=False)
    wv_d = nc.declare_dram_parameter("wv", [128, NCH, NCH, 128], F32, isOutput=False)
    wo_d = nc.declare_dram_parameter("wo", [128, NCH, NCH, 128], F32, isOutput=False)
    c1w_d = nc.declare_dram_parameter("c1w", [128, NCH, NCH, 128], F32, isOutput=False)
    w1_d = nc.declare_dram_parameter("w1", [128, NF, NCH, 128], F32, isOutput=False)
    w2_d = nc.declare_dram_parameter("w2", [128, NF, NCH, 128], F32, isOutput=False)
    scal_d = nc.declare_dram_parameter("scal", [128, NSCAL], F32, isOutput=False)
    b1c_d = nc.declare_dram_parameter("b1c", [128, NF], F32, isOutput=False)
    clf2_d = nc.declare_dram_parameter("clf2", [128, NCH], F32, isOutput=False)

    h2o_d = nc.declare_dram_parameter("h2o", [NCH, 128, NCOL], F32, isOutput=True)
    po_d = nc.declare_dram_parameter("po", [1, NCOL], F32, isOutput=True)

    with tile.TileContext(nc) as tc:
        with (
            tc.tile_pool(name="w", bufs=1) as wp,
            tc.tile_pool(name="act", bufs=1) as ap,
            tc.tile_pool(name="rp", bufs=4) as rp,
            tc.tile_pool(name="psA", bufs=3, space="PSUM") as psA,
            tc.tile_pool(name="psB", bufs=2, space="PSUM") as psB,
            tc.tile_pool(name="psC", bufs=2, space="PSUM") as psC,
            tc.tile_pool(name="psD", bufs=1, space="PSUM") as psD,
        ):
            # ---- weight/const loads (issued in consumption order) ----
            xt = wp.tile([128, NCH, NCOL], F32, tag="xt")
            nc.sync.dma_start(xt[:], xt_d[:])
            wtilde = wp.tile([128, NCH, H], F32, tag="wtilde")
            nc.sync.dma_start(wtilde[:], wtilde_d[:])
            sm4 = wp.tile([H, 2], F32, tag="sm4")
            nc.sync.dma_start(sm4[:], sm4_d[:])
            hb = wp.tile([H, NCH, 128], F32, tag="hb")
            nc.sync.dma_start(hb[:], hb_d[:])
            wv = wp.tile([128, NCH, NCH, 128], F32, tag="wv")
            nc.sync.dma_start(wv[:], wv_d[:])
            scal = wp.tile([128, NSCAL], F32, tag="scal")
            nc.sync.dma_start(scal[:], scal_d[:])
            wo = wp.tile([128, NCH, NCH, 128], F32, tag="wo")
            nc.sync.dma_start(wo[:], wo_d[:])
            w1 = wp.tile([128, NF, NCH, 128], F32, tag="w1")
            for fg in range(4):
                nc.sync.dma_start(w1[:, 4 * fg:4 * fg + 4, :, :],
                                  w1_d[:, 4 * fg:4 * fg + 4, :, :])
            w2 = wp.tile([128, NF, NCH, 128], F32, tag="w2")
            for fg in range(4):
                nc.sync.dma_start(w2[:, 4 * fg:4 * fg + 4, :, :],
                                  w2_d[:, 4 * fg:4 * fg + 4, :, :])
            b1c = wp.tile([128, NF], F32, tag="b1c")
            nc.sync.dma_start(b1c[:], b1c_d[:])
            c1w = wp.tile([128, NCH, NCH, 128], F32, tag="c1w")
            nc.sync.dma_start(c1w[:], c1w_d[:])
            clf2 = wp.tile([128, NCH], F32, tag="clf2")
            nc.sync.dma_start(clf2[:], clf2_d[:])

            ones = wp.tile([128, 1], F32, tag="ones")
            nc.gpsimd.memset(ones[:], 1.0)
            ones1 = wp.tile([1, 128], F32, tag="ones1")
            nc.gpsimd.memset(ones1[:], 1.0)
            cst = wp.tile([1, 2], F32, tag="cst")
            nc.gpsimd.memset(cst[:, 0:1], EPS)
            nc.gpsimd.memset(cst[:, 1:2], float(c2_const))

            # ---- attention: scores -> e -> V -> prefix sums -> o0 ----
            ps_sc = psC.tile([H, NCOL], F32, tag="small")
            for kc in range(NCH):
                nc.tensor.matmul(ps_sc[:], wtilde[:, kc, :], xt[:, kc, :],
                                 start=(kc == 0), stop=(kc == NCH - 1))
            e_sb = ap.tile([H, NCOL], F32, tag="e_sb")
            nc.scalar.activation(e_sb[:], ps_sc[:], AF.Exp, bias=sm4[:, 0:1])

            v_sb, ev_a, ev_b = [], [], []
            for mc in range(NCH):
                ps_v = psA.tile([128, NCOL], F32, tag="t")
                for kc in range(NCH):
                    nc.tensor.matmul(ps_v[:], wv[:, kc, mc, :], xt[:, kc, :],
                                     start=(kc == 0), stop=(kc == NCH - 1))
                vt = ap.tile([128, NCOL], F32, tag=f"v{mc}")
                nc.scalar.activation(vt[:], ps_v[:], AF.Identity,
                                     bias=scal[:, S_BV + mc:S_BV + mc + 1])
                v_sb.append(vt)
                ev_a.append(ap.tile([128, NCOL], F32, tag=f"eva{mc}", name=f"eva{mc}"))
                ev_b.append(ap.tile([128, NCOL], F32, tag=f"evb{mc}", name=f"evb{mc}"))

            for mc in range(NCH):
                ps_ef = psA.tile([128, NCOL], F32, tag="t")
                nc.tensor.matmul(ps_ef[:], hb[:, mc, :], e_sb[:])
                nc.vector.tensor_mul(ev_a[mc][:], v_sb[mc][:], ps_ef[:])

            # segmented (per-problem) inclusive prefix sums along tokens:
            # col = j*PPC + p, shift by s tokens = s*PPC columns
            for mc in range(NCH):
                a, b = ev_a[mc], ev_b[mc]
                for s in (1, 2, 4, 8):
                    w_ = s * PPC
                    nc.vector.tensor_add(b[:, w_:NCOL], a[:, w_:NCOL], a[:, 0:NCOL - w_])
                    nc.gpsimd.tensor_copy(b[:, 0:w_], a[:, 0:w_])
                    a, b = b, a
                assert a is ev_a[mc]

            et = ap.tile([H, NCOL], F32, tag="et")
            a, b = e_sb, et
            for s in (1, 2):
                w_ = s * PPC
                nc.vector.tensor_add(b[:, w_:NCOL], a[:, w_:NCOL], a[:, 0:NCOL - w_])
                nc.gpsimd.tensor_copy(b[:, 0:w_], a[:, 0:w_])
                a, b = b, a
            # s=4 then s=8 fused with +e0 on the final step
            w_ = 4 * PPC
            nc.vector.tensor_add(b[:, w_:NCOL], a[:, w_:NCOL], a[:, 0:NCOL - w_])
            nc.gpsimd.tensor_copy(b[:, 0:w_], a[:, 0:w_])
            a, b = b, a
            w_ = 8 * PPC
            nc.vector.scalar_tensor_tensor(
                b[:, w_:NCOL], a[:, w_:NCOL], sm4[:, 1:2], a[:, 0:NCOL - w_],
                op0=ALU.add, op1=ALU.add)
            nc.vector.tensor_scalar_add(b[:, 0:w_], a[:, 0:w_], sm4[:, 1:2])
            cume = b

            o0 = []
            for mc in range(NCH):
                ps_df = psA.tile([128, NCOL], F32, tag="t")
                nc.tensor.matmul(ps_df[:], hb[:, mc, :], cume[:])
                rc = ap.tile([128, NCOL], F32, tag=f"rc{mc}")
                nc.vector.reciprocal(rc[:], ps_df[:])
                ot = ap.tile([128, NCOL], F32, tag=f"o0{mc}")
                nc.vector.scalar_tensor_tensor(
                    ot[:], ev_a[mc][:], scal[:, S_INIT + mc:S_INIT + mc + 1], rc[:],
                    op0=ALU.add, op1=ALU.mult)
                o0.append(ot)

            # ---- out-proj + residual(cls) ----
            pre1 = []
            for mc in range(NCH):
                ps_at = psA.tile([128, NCOL], F32, tag="t")
                for kc in range(NCH):
                    nc.tensor.matmul(ps_at[:], wo[:, kc, mc, :], o0[kc][:],
                                     start=(kc == 0), stop=(kc == NCH - 1))
                pt = ap.tile([128, NCOL], F32, tag=f"pre1{mc}")
                nc.scalar.activation(pt[:], ps_at[:], AF.Identity,
                                     bias=scal[:, S_BOCLS + mc:S_BOCLS + mc + 1])
                pre1.append(pt)

            def layernorm(src, wcol, bcol, out_tag):
                """LN over the feature dim (partition chunks) of src[2][128,NCOL]."""
                ps_sum = psC.tile([1, NCOL], F32, tag="small")
                ps_sq = psC.tile([1, NCOL], F32, tag="small")
                sq = []
                for kc in range(NCH):
                    st = ap.tile([128, NCOL], F32, tag=f"sq{kc}")
                    nc.gpsimd.tensor_mul(st[:], src[kc][:], src[kc][:])
                    sq.append(st)
                for kc in range(NCH):
                    nc.tensor.matmul(ps_sum[:], ones[:], src[kc][:],
                                     start=(kc == 0), stop=(kc == NCH - 1))
                for kc in range(NCH):
                    nc.tensor.matmul(ps_sq[:], ones[:], sq[kc][:],
                                     start=(kc == 0), stop=(kc == NCH - 1))
                bsrc = ap.tile([1, 2 * NCOL], F32, tag="bsrc")
                m = bsrc[:, 0:NCOL]
                nc.scalar.activation(m, ps_sum[:], AF.Copy, scale=1.0 / D)
                e2 = ap.tile([1, NCOL], F32, tag="e2")
                nc.scalar.activation(e2[:], ps_sq[:], AF.Copy, scale=1.0 / D)
                var = ap.tile([1, NCOL], F32, tag="var")
                # var = e2 - m*m  ->  (m mult m) subtract-reversed e2 is not
                # available; use two ops
                nc.vector.tensor_mul(var[:], m, m)
                nc.vector.tensor_sub(var[:], e2[:], var[:])
                sd = ap.tile([1, NCOL], F32, tag="sd")
                nc.scalar.activation(sd[:], var[:], AF.Sqrt, bias=cst[:, 0:1])
                nc.vector.reciprocal(bsrc[:, NCOL:2 * NCOL], sd[:])
                ps_bc = psD.tile([128, 2 * NCOL], F32, tag="bc")
                nc.tensor.matmul(ps_bc[:], ones1[:], bsrc[:])
                outs = []
                for kc in range(NCH):
                    u = ap.tile([128, NCOL], F32, tag=f"{out_tag}{kc}")
                    nc.vector.tensor_sub(u[:], src[kc][:], ps_bc[:, 0:NCOL])
                    nc.vector.tensor_mul(u[:], u[:], ps_bc[:, NCOL:2 * NCOL])
                    nc.vector.tensor_scalar(
                        u[:], u[:], scal[:, wcol + kc:wcol + kc + 1],
                        scal[:, bcol + kc:bcol + kc + 1],
                        op0=ALU.mult, op1=ALU.add)
                    outs.append(u)
                return outs

            h1 = layernorm(pre1, S_LN1W, S_LN1B, "h1_")

            # ---- FF: 256 -> 2048 -> 256, interleaved accumulate ----
            ps_h2 = [psB.tile([128, NCOL], F32, tag="acc", name=f"ps_h2_{mc}") for mc in range(NCH)]
            for f in range(NF):
                ps_f = psA.tile([128, NCOL], F32, tag="t")
                for kc in range(NCH):
                    nc.tensor.matmul(ps_f[:], w1[:, f, kc, :], h1[kc][:],
                                     start=(kc == 0), stop=(kc == NCH - 1))
                r = rp.tile([128, NCOL], F32, tag="r")
                if f % 2 == 0:
                    nc.scalar.activation(r[:], ps_f[:], AF.Relu,
                                         bias=b1c[:, f:f + 1])
                else:
                    nc.vector.tensor_scalar(
                        r[:], ps_f[:], b1c[:, f:f + 1], 0.0,
                        op0=ALU.add, op1=ALU.max)
                for mc in range(NCH):
                    nc.tensor.matmul(ps_h2[mc][:], w2[:, f, mc, :], r[:],
                                     start=(f == 0), stop=(f == NF - 1))

            pre2 = []
            for mc in range(NCH):
                pt = ap.tile([128, NCOL], F32, tag=f"pre2{mc}")
                nc.vector.scalar_tensor_tensor(
                    pt[:], ps_h2[mc][:], scal[:, S_B2 + mc:S_B2 + mc + 1],
                    h1[mc][:], op0=ALU.add, op1=ALU.add)
                pre2.append(pt)

            h2 = layernorm(pre2, S_LN2W, S_LN2B, "h2_")
            for mc in range(NCH):
                nc.sync.dma_start(h2o_d[mc], h2[mc][:])

            # ---- classifier ----
            ps_l = psC.tile([1, NCOL], F32, tag="small")
            rcl = []
            for mc in range(NCH):
                ps_c = psA.tile([128, NCOL], F32, tag="t")
                for kc in range(NCH):
                    nc.tensor.matmul(ps_c[:], c1w[:, kc, mc, :], h2[kc][:],
                                     start=(kc == 0), stop=(kc == NCH - 1))
                rt = ap.tile([128, NCOL], F32, tag=f"rcl{mc}")
                nc.scalar.activation(rt[:], ps_c[:], AF.Relu,
                                     bias=scal[:, S_C1 + mc:S_C1 + mc + 1])
                rcl.append(rt)
            for kc in range(NCH):
                nc.tensor.matmul(ps_l[:], clf2[:, kc:kc + 1], rcl[kc][:],
                                 start=(kc == 0), stop=(kc == NCH - 1))
            p_sb = ap.tile([1, NCOL], F32, tag="p_sb")
            nc.scalar.activation(p_sb[:], ps_l[:], AF.Sigmoid, bias=cst[:, 1:2])
            nc.sync.dma_start(po_d[:], p_sb[:])

    nc.compile()
    return nc


_CACHE = {}


def _prep_host(inputs):
    f32 = np.float32
    g = lambda k: np.ascontiguousarray(np.asarray(inputs[k], f32))
    frames = g('frames')
    cls = g('cls_token')
    Wqkv, bqkv = g('in_proj_w'), g('in_proj_b')
    Wq, Wk, Wv = Wqkv[:D], Wqkv[D:2 * D], Wqkv[2 * D:]
    bq, bk, bv = bqkv[:D], bqkv[D:2 * D], bqkv[2 * D:]
    Wo, bo = g('out_proj_w'), g('out_proj_b')
    ln1w, ln1b = g('ln1_w'), g('ln1_b')
    W1, b1 = g('lin1_w'), g('lin1_b')
    W2, b2 = g('lin2_w'), g('lin2_b')
    ln2w, ln2b = g('ln2_w'), g('ln2_b')
    C1, c1 = g('clf1_w'), g('clf1_b')
    C2, c2 = g('clf2_w'), np.float32(np.asarray(inputs['clf2_b']))

    q0 = Wq @ cls + bq
    q0h = q0.reshape(H, HD)
    Wtilde = (np.einsum('hd,hdD->Dh', q0h, Wk.reshape(H, HD, D)) * SCALE).astype(f32)
    c_h = (np.einsum('hd,hd->h', q0h, bk.reshape(H, HD)) * SCALE).astype(f32)
    k_cls = (Wk @ cls + bk).reshape(H, HD)
    s0 = (np.einsum('hd,hd->h', q0h, k_cls) * SCALE).astype(f32)
    e0 = np.exp(s0)
    v_cls = Wv @ cls + bv
    init = (np.repeat(e0, HD) * v_cls).astype(f32)          # (D,)

    frames_pad = np.concatenate([frames, np.zeros((B, K, D), f32)], axis=1)
    idx = np.arange(T)[:, None] + np.arange(K)[None, :]
    Xw = frames_pad[:, idx, :].reshape(B * T, K, D)          # (128, 16, 256)

    xt_cores = []
    for c in range(NCORES):
        blk = Xw[c * PPC:(c + 1) * PPC]                      # (16, 16, 256) [p, j, d]
        xt = np.ascontiguousarray(
            blk.transpose(2, 1, 0).reshape(NCH, 128, NCOL).transpose(1, 0, 2))
        xt_cores.append(xt)

    tchunks = lambda M: np.ascontiguousarray(                # Wt -> [128, kc, mc, 128]
        M.T.reshape(NCH, 128, NCH, 128).transpose(1, 0, 2, 3))
    wv_h = tchunks(Wv)
    wo_h = tchunks(Wo)
    c1w_h = tchunks(C1)
    w1_h = np.ascontiguousarray(                             # [128, f, kc, 128]
        W1.T.reshape(NCH, 128, NF, 128).transpose(1, 2, 0, 3))
    w2_h = np.ascontiguousarray(                             # [128, f, mc, 128]
        W2.T.reshape(NF, 128, NCH, 128).transpose(1, 0, 2, 3))

    scal = np.zeros((128, NSCAL), f32)
    pc = lambda v: v.reshape(NCH, 128).T                     # (128, 2)
    scal[:, S_BOCLS:S_BOCLS + 2] = pc(bo + cls)
    scal[:, S_BV:S_BV + 2] = pc(bv)
    scal[:, S_LN1W:S_LN1W + 2] = pc(ln1w)
    scal[:, S_LN1B:S_LN1B + 2] = pc(ln1b)
    scal[:, S_LN2W:S_LN2W + 2] = pc(ln2w)
    scal[:, S_LN2B:S_LN2B + 2] = pc(ln2b)
    scal[:, S_B2:S_B2 + 2] = pc(b2)
    scal[:, S_C1:S_C1 + 2] = pc(c1)
    scal[:, S_INIT:S_INIT + 2] = pc(init)

    sm4 = np.stack([c_h, e0], axis=1).astype(f32)            # (4, 2)
    hb_h = np.zeros((H, NCH, 128), f32)
    for h in range(H):
        for mc in range(NCH):
            for m in range(128):
                if (mc * 128 + m) // HD == h:
                    hb_h[h, mc, m] = 1.0
    b1c_h = np.ascontiguousarray(b1.reshape(NF, 128).T)
    clf2_h = np.ascontiguousarray(C2.reshape(NCH, 128).T)
    wtilde_h = np.ascontiguousarray(Wtilde.reshape(NCH, 128, H).transpose(1, 0, 2))

    common = dict(wtilde=wtilde_h, sm4=sm4, hb=hb_h, wv=wv_h, wo=wo_h,
                  c1w=c1w_h, w1=w1_h, w2=w2_h, scal=scal, b1c=b1c_h,
                  clf2=clf2_h)
    in_maps = [dict(common, xt=xt_cores[c]) for c in range(NCORES)]
    return in_maps, float(c2)


def kernel(**inputs):
    in_maps, c2 = _prep_host(inputs)
    key = 'nc'
    if key not in _CACHE or _CACHE.get('c2') != c2:
        _CACHE[key] = _build_nc(c2)
        _CACHE['c2'] = c2
    nc = _CACHE[key]

    res = run_bass_kernel_spmd(nc, in_maps, list(range(NCORES)))

    # gather: per core h2o [2, 128, 256] -> H2[b, s, L, :], po -> P[b, s, L]
    H2 = np.zeros((B * T, K, D), np.float32)
    P = np.zeros((B * T, K), np.float32)
    for c in range(NCORES):
        h2o = res.results[c]['h2o']                # (2, 128, NCOL)
        po = res.results[c]['po'].reshape(NCOL)    # (NCOL,)
        h2r = h2o.reshape(NCH, 128, K, PPC)        # [kc, d, j, p]
        h2r = h2r.transpose(3, 2, 0, 1).reshape(PPC, K, D)
        H2[c * PPC:(c + 1) * PPC] = h2r
        P[c * PPC:(c + 1) * PPC] = po.reshape(K, PPC).T
    H2 = H2.reshape(B, T, K, D)
    P = P.reshape(B, T, K)

    lengths = np.arange(1, K + 1)
    out = np.zeros((B, T, D), np.float32)
    for b in range(B):
        start, cnt = 0, 0
        while start < T:
            limit = min(K, T - start)
            stop = (P[b, start] >= THR) & (lengths <= limit)
            i = int(np.argmax(stop)) if stop.any() else limit - 1
            out[b, cnt] = H2[b, start, i]
            start += int(lengths[i])
            cnt += 1
    return out
